# revision 14
# baseline (speedup 1.0000x reference)
"""AttentionAggregation GNN message passing on 8 trn2 NeuronCores.

Sharding: destinations are split across the 8 cores (49 blocks of 128 dst
slots per core, host-balanced so every block has nearly equal edge load).
Each core gathers the source-feature rows (bf16) for its edges from the
replicated node table with indirect DMA, applies sigmoid attention, and
scatter-adds into per-block [128 dst, 128 feat + 1 count] PSUM accumulators
via one-hot matmuls (the one-hot scatter matrix is built on the vector
engine with a fused is_equal*att tensor_scalar).  The host divides by the
count and unpermutes.
"""

import sys

for _p in ("/opt/trn_rl_repo", "/root/.axon_site/_ro/trn_rl_repo"):
    if _p not in sys.path:
        sys.path.append(_p)

import numpy as np
import ml_dtypes

N_NODES = 50000
D = 128
N_GRP = 8           # dst groups (one per core)
SLOTS = 49          # dst blocks per core (49 * 128 = 6272 >= 6250)
P = 128
PAD_OFF = 384.0     # dst_off sentinel for padding edges (no iota match)

_COMPILED = {}


def _assign_bins(deg):
    """Assign each dst node to one of 8*SLOTS bins (<=128 dsts each),
    balancing total edge load per bin."""
    import heapq

    nbins = N_GRP * SLOTS
    order = np.argsort(-deg, kind="stable")
    load = np.zeros(nbins, np.int64)
    count = np.zeros(nbins, np.int64)
    bin_of = np.empty(N_NODES, np.int32)
    pos_of = np.empty(N_NODES, np.int32)
    heap = [(0, b) for b in range(nbins)]
    heapq.heapify(heap)
    for v in order:
        key, b = heapq.heappop(heap)
        bin_of[v] = b
        pos_of[v] = count[b]
        count[b] += 1
        load[b] += deg[v]
        if count[b] < P:
            heapq.heappush(heap, (load[b], b))
    return bin_of, pos_of, load


def _preprocess(src_feat, dst_feat, att_w, att_b, edge_index):
    src = np.asarray(edge_index[0], dtype=np.int64)
    dst = np.asarray(edge_index[1], dtype=np.int64)
    E = src.shape[0]

    a = (src_feat @ att_w[:D, 0]).astype(np.float32)
    b = (dst_feat @ att_w[D:, 0] + np.float32(att_b[0])).astype(np.float32)

    deg = np.bincount(dst, minlength=N_NODES)
    bin_of, pos_of, load = _assign_bins(deg)

    # Group the 392 bins into SLOTS slots of 8 (one bin per core), similar
    # loads together so the shared per-slot chunk count C_j is tight.
    bin_order = np.argsort(-load, kind="stable")
    slot_of_bin = np.empty(N_GRP * SLOTS, np.int32)
    grp_of_bin = np.empty(N_GRP * SLOTS, np.int32)
    C = np.empty(SLOTS, np.int64)
    for s in range(SLOTS):
        grp = bin_order[s * N_GRP:(s + 1) * N_GRP]
        slot_of_bin[grp] = s
        grp_of_bin[grp] = np.arange(N_GRP)
        C[s] = max(1, -(-int(load[grp].max()) // P))

    NB = int(C.sum())
    blk_base = np.zeros(SLOTS, np.int64)
    blk_base[1:] = np.cumsum(C)[:-1]

    e_bin = bin_of[dst]
    e_core = grp_of_bin[e_bin]
    e_slot = slot_of_bin[e_bin]
    e_off = pos_of[dst]
    e_c = a[src] + b[dst]

    # within-(core,slot) rank
    key = e_core.astype(np.int64) * SLOTS + e_slot
    sort_idx = np.argsort(key, kind="stable")
    ks = key[sort_idx]
    new_grp = np.ones(E, bool)
    new_grp[1:] = ks[1:] != ks[:-1]
    grp_ids = np.cumsum(new_grp) - 1
    first_pos = np.zeros(grp_ids[-1] + 1, np.int64)
    first_pos[grp_ids[new_grp]] = np.nonzero(new_grp)[0]
    rank_sorted = np.arange(E) - first_pos[grp_ids]
    rank = np.empty(E, np.int64)
    rank[sort_idx] = rank_sorted

    if np.any(rank >= (C * P)[e_slot]):
        raise RuntimeError("bin packing overflow")
    stream_pos = blk_base[e_slot] * P + rank

    return dict(
        NB=NB, C=C,
        e_core=e_core, stream_pos=stream_pos, e_src=src,
        e_off=e_off, e_c=e_c,
        bin_of=bin_of, pos_of=pos_of, grp_of_bin=grp_of_bin,
        slot_of_bin=slot_of_bin,
    )


def _build_core_inputs(pre, src_feat, CHB):
    NB = pre["NB"]
    NBpad = -(-NB // CHB) * CHB
    EPC = NBpad * P

    idx32 = np.zeros((8, EPC), np.int32)
    dstoff = np.full((8, EPC), PAD_OFF, np.float32)
    cval = np.zeros((8, EPC), np.float32)

    ec = pre["e_core"]
    sp = pre["stream_pos"]
    idx32[ec, sp] = pre["e_src"].astype(np.int32)
    dstoff[ec, sp] = pre["e_off"].astype(np.float32)
    cval[ec, sp] = pre["e_c"]

    table = np.ascontiguousarray(src_feat.astype(ml_dtypes.bfloat16))
    iota = np.tile(np.arange(P, dtype=np.float32), (P, 1)).astype(ml_dtypes.bfloat16)
    ones = np.ones((P, 1), ml_dtypes.bfloat16)

    in_maps = []
    for c in range(8):
        in_maps.append({
            "table": table,
            # [lane, chunk] layout: offsets tile element [p, j] = edge
            # (chunk j, lane p)
            "idx": idx32[c].reshape(NBpad, P).T.copy(),
            "dstoff": dstoff[c].reshape(NBpad, P).T.copy(),
            "cval": cval[c].reshape(NBpad, P).T.copy(),
            "iota": iota,
            "ones": ones,
        })
    return in_maps, NBpad


def _build_kernel(C, NBpad, CHB, gbufs=8, lbufs=8, psbufs=4):
    import concourse.bass as bass
    import concourse.bacc as bacc
    import concourse.tile as tile
    import concourse.mybir as mybir
    from contextlib import ExitStack

    f32 = mybir.dt.float32
    bf16 = mybir.dt.bfloat16

    nc = bacc.Bacc("TRN2", target_bir_lowering=False, debug=False)
    table = nc.dram_tensor("table", [N_NODES, D], bf16, kind="ExternalInput")
    idx_h = nc.dram_tensor("idx", [P, NBpad], mybir.dt.int32, kind="ExternalInput")
    dstoff_h = nc.dram_tensor("dstoff", [P, NBpad], f32, kind="ExternalInput")
    cval_h = nc.dram_tensor("cval", [P, NBpad], f32, kind="ExternalInput")
    iota_h = nc.dram_tensor("iota", [P, P], bf16, kind="ExternalInput")
    ones_h = nc.dram_tensor("ones", [P, 1], bf16, kind="ExternalInput")
    out_h = nc.dram_tensor("out", [SLOTS, P, D + 1], f32, kind="ExternalOutput")

    with tile.TileContext(nc) as tc, ExitStack() as ctx:
        const = ctx.enter_context(tc.tile_pool(name="const", bufs=1))
        gpool = ctx.enter_context(tc.tile_pool(name="g", bufs=gbufs))
        lpool = ctx.enter_context(tc.tile_pool(name="lh", bufs=lbufs))
        pspool = ctx.enter_context(tc.tile_pool(name="ps", bufs=psbufs, space="PSUM"))
        opool = ctx.enter_context(tc.tile_pool(name="o", bufs=4))

        idx_sb = const.tile([P, NBpad], mybir.dt.int32)
        dstoff_sb = const.tile([P, NBpad], f32)
        cval_sb = const.tile([P, NBpad], f32)
        att_sb = const.tile([P, NBpad], f32)
        iota_sb = const.tile([P, P], bf16)
        ones_sb = const.tile([P, 1], bf16)
        nc.sync.dma_start(idx_sb[:], idx_h[:])
        nc.sync.dma_start(dstoff_sb[:], dstoff_h[:])
        nc.sync.dma_start(cval_sb[:], cval_h[:])
        nc.sync.dma_start(iota_sb[:], iota_h[:])
        nc.sync.dma_start(ones_sb[:], ones_h[:])
        nc.scalar.activation(att_sb[:], cval_sb[:],
                             mybir.ActivationFunctionType.Sigmoid)

        chunk = 0
        for j in range(SLOTS):
            Cj = int(C[j])
            ps = pspool.tile([P, D + 1], f32, tag="ps")
            for t in range(Cj):
                gt = gpool.tile([P, D], bf16, tag="g")
                nc.gpsimd.indirect_dma_start(
                    out=gt[:],
                    out_offset=None,
                    in_=table[:],
                    in_offset=bass.IndirectOffsetOnAxis(
                        ap=idx_sb[:, chunk:chunk + 1], axis=0),
                )
                lh = lpool.tile([P, P], bf16, tag="lh")
                nc.vector.tensor_scalar(
                    lh[:], iota_sb[:],
                    dstoff_sb[:, chunk:chunk + 1],
                    att_sb[:, chunk:chunk + 1],
                    op0=mybir.AluOpType.is_equal,
                    op1=mybir.AluOpType.mult)
                nc.tensor.matmul(ps[:, 0:D], lh[:], gt[:],
                                 start=(t == 0), stop=False)
                nc.tensor.matmul(ps[:, D:D + 1], lh[:], ones_sb[:],
                                 start=False, stop=(t == Cj - 1))
                chunk += 1
            ot = opool.tile([P, D + 1], f32, tag="ot")
            nc.scalar.copy(ot[:], ps[:])
            nc.sync.dma_start(out_h[j], ot[:])
    nc.compile()
    return nc


def kernel(src_feat, dst_feat, att_w, att_b, edge_index, n_dst):
    from concourse.bass_utils import run_bass_kernel_spmd

    src_feat = np.asarray(src_feat, dtype=np.float32)
    dst_feat = np.asarray(dst_feat, dtype=np.float32)
    att_w = np.asarray(att_w, dtype=np.float32)
    att_b = np.asarray(att_b, dtype=np.float32)
    n_dst = int(n_dst)
    assert src_feat.shape == (N_NODES, D) and n_dst == N_NODES

    CHB = 8
    pre = _preprocess(src_feat, dst_feat, att_w, att_b, edge_index)
    in_maps, NBpad = _build_core_inputs(pre, src_feat, CHB)

    key = (tuple(pre["C"].tolist()), NBpad, CHB)
    if key not in _COMPILED:
        _COMPILED[key] = _build_kernel(pre["C"], NBpad, CHB)
    nc = _COMPILED[key]

    res = run_bass_kernel_spmd(nc, in_maps, core_ids=list(range(8)))
    outs = [res.results[c]["out"] for c in range(8)]  # [SLOTS,128,129] f32

    bin_of = pre["bin_of"]
    grp = pre["grp_of_bin"][bin_of]
    slot = pre["slot_of_bin"][bin_of]
    pos = pre["pos_of"]
    agg = np.empty((N_NODES, D), np.float32)
    cnt = np.empty(N_NODES, np.float32)
    for g in range(8):
        m = grp == g
        o = outs[g]
        agg[m] = o[slot[m], pos[m], :D]
        cnt[m] = o[slot[m], pos[m], D]
    cnt = np.maximum(cnt, np.float32(1e-8))
    return (agg / cnt[:, None]).astype(np.float32)


# revision 15
# speedup vs baseline: 1.0063x; 1.0063x over previous
"""AttentionAggregation GNN message passing on 8 trn2 NeuronCores.

Sharding: destinations are split across the 8 cores (49 blocks of 128 dst
slots per core, host-balanced so every block has nearly equal edge load).
Each core gathers the source-feature rows (bf16) for its edges from the
replicated node table with indirect DMA, applies sigmoid attention, and
scatter-adds into per-block [128 dst, 128 feat + 1 count] PSUM accumulators
via one-hot matmuls (the one-hot scatter matrix is built on the vector
engine with a fused is_equal*att tensor_scalar).  The host divides by the
count and unpermutes.
"""

import sys

for _p in ("/opt/trn_rl_repo", "/root/.axon_site/_ro/trn_rl_repo"):
    if _p not in sys.path:
        sys.path.append(_p)

import numpy as np
import ml_dtypes

N_NODES = 50000
D = 128
N_GRP = 8           # dst groups (one per core)
SLOTS = 49          # dst blocks per core (49 * 128 = 6272 >= 6250)
P = 128
PAD_OFF = 384.0     # dst_off sentinel for padding edges (no iota match)

_COMPILED = {}


def _assign_bins(deg):
    """Assign each dst node to one of 8*SLOTS bins (<=128 dsts each),
    balancing total edge load per bin."""
    import heapq

    nbins = N_GRP * SLOTS
    order = np.argsort(-deg, kind="stable")
    load = np.zeros(nbins, np.int64)
    count = np.zeros(nbins, np.int64)
    bin_of = np.empty(N_NODES, np.int32)
    pos_of = np.empty(N_NODES, np.int32)
    heap = [(0, b) for b in range(nbins)]
    heapq.heapify(heap)
    for v in order:
        key, b = heapq.heappop(heap)
        bin_of[v] = b
        pos_of[v] = count[b]
        count[b] += 1
        load[b] += deg[v]
        if count[b] < P:
            heapq.heappush(heap, (load[b], b))
    return bin_of, pos_of, load


def _preprocess(src_feat, dst_feat, att_w, att_b, edge_index):
    src = np.asarray(edge_index[0], dtype=np.int64)
    dst = np.asarray(edge_index[1], dtype=np.int64)
    E = src.shape[0]

    a = (src_feat @ att_w[:D, 0]).astype(np.float32)
    b = (dst_feat @ att_w[D:, 0] + np.float32(att_b[0])).astype(np.float32)

    deg = np.bincount(dst, minlength=N_NODES)
    bin_of, pos_of, load = _assign_bins(deg)

    # Group the 392 bins into SLOTS slots of 8 (one bin per core), similar
    # loads together so the shared per-slot chunk count C_j is tight.
    bin_order = np.argsort(-load, kind="stable")
    slot_of_bin = np.empty(N_GRP * SLOTS, np.int32)
    grp_of_bin = np.empty(N_GRP * SLOTS, np.int32)
    C = np.empty(SLOTS, np.int64)
    for s in range(SLOTS):
        grp = bin_order[s * N_GRP:(s + 1) * N_GRP]
        slot_of_bin[grp] = s
        grp_of_bin[grp] = np.arange(N_GRP)
        C[s] = max(1, -(-int(load[grp].max()) // P))

    NB = int(C.sum())
    blk_base = np.zeros(SLOTS, np.int64)
    blk_base[1:] = np.cumsum(C)[:-1]

    e_bin = bin_of[dst]
    e_core = grp_of_bin[e_bin]
    e_slot = slot_of_bin[e_bin]
    e_off = pos_of[dst]
    e_c = a[src] + b[dst]

    # within-(core,slot) rank
    key = e_core.astype(np.int64) * SLOTS + e_slot
    sort_idx = np.argsort(key, kind="stable")
    ks = key[sort_idx]
    new_grp = np.ones(E, bool)
    new_grp[1:] = ks[1:] != ks[:-1]
    grp_ids = np.cumsum(new_grp) - 1
    first_pos = np.zeros(grp_ids[-1] + 1, np.int64)
    first_pos[grp_ids[new_grp]] = np.nonzero(new_grp)[0]
    rank_sorted = np.arange(E) - first_pos[grp_ids]
    rank = np.empty(E, np.int64)
    rank[sort_idx] = rank_sorted

    if np.any(rank >= (C * P)[e_slot]):
        raise RuntimeError("bin packing overflow")
    stream_pos = blk_base[e_slot] * P + rank

    return dict(
        NB=NB, C=C,
        e_core=e_core, stream_pos=stream_pos, e_src=src,
        e_off=e_off, e_c=e_c,
        bin_of=bin_of, pos_of=pos_of, grp_of_bin=grp_of_bin,
        slot_of_bin=slot_of_bin,
    )


def _build_core_inputs(pre, src_feat, CHB):
    NB = pre["NB"]
    NBpad = -(-NB // CHB) * CHB
    EPC = NBpad * P

    idx32 = np.zeros((8, EPC), np.int32)
    dstoff = np.full((8, EPC), PAD_OFF, np.float32)
    cval = np.zeros((8, EPC), np.float32)

    ec = pre["e_core"]
    sp = pre["stream_pos"]
    idx32[ec, sp] = pre["e_src"].astype(np.int32)
    dstoff[ec, sp] = pre["e_off"].astype(np.float32)
    cval[ec, sp] = pre["e_c"]

    table = np.ascontiguousarray(src_feat.astype(ml_dtypes.bfloat16))
    iota = np.tile(np.arange(P, dtype=np.float32), (P, 1)).astype(ml_dtypes.bfloat16)
    ones = np.ones((P, 1), ml_dtypes.bfloat16)

    in_maps = []
    for c in range(8):
        in_maps.append({
            "table": table,
            # [lane, chunk] layout: offsets tile element [p, j] = edge
            # (chunk j, lane p)
            "idx": idx32[c].reshape(NBpad, P).T.copy(),
            "dstoff": dstoff[c].reshape(NBpad, P).T.copy(),
            "cval": cval[c].reshape(NBpad, P).T.copy(),
            "iota": iota,
            "ones": ones,
        })
    return in_maps, NBpad


def _build_kernel(C, NBpad, CHB, gbufs=14, lbufs=14, psbufs=6):
    import concourse.bass as bass
    import concourse.bacc as bacc
    import concourse.tile as tile
    import concourse.mybir as mybir
    from contextlib import ExitStack

    f32 = mybir.dt.float32
    bf16 = mybir.dt.bfloat16

    nc = bacc.Bacc("TRN2", target_bir_lowering=False, debug=False)
    table = nc.dram_tensor("table", [N_NODES, D], bf16, kind="ExternalInput")
    idx_h = nc.dram_tensor("idx", [P, NBpad], mybir.dt.int32, kind="ExternalInput")
    dstoff_h = nc.dram_tensor("dstoff", [P, NBpad], f32, kind="ExternalInput")
    cval_h = nc.dram_tensor("cval", [P, NBpad], f32, kind="ExternalInput")
    iota_h = nc.dram_tensor("iota", [P, P], bf16, kind="ExternalInput")
    ones_h = nc.dram_tensor("ones", [P, 1], bf16, kind="ExternalInput")
    out_h = nc.dram_tensor("out", [SLOTS, P, D + 1], f32, kind="ExternalOutput")

    with tile.TileContext(nc) as tc, ExitStack() as ctx:
        const = ctx.enter_context(tc.tile_pool(name="const", bufs=1))
        gpool = ctx.enter_context(tc.tile_pool(name="g", bufs=gbufs))
        lpool = ctx.enter_context(tc.tile_pool(name="lh", bufs=lbufs))
        pspool = ctx.enter_context(tc.tile_pool(name="ps", bufs=psbufs, space="PSUM"))
        opool = ctx.enter_context(tc.tile_pool(name="o", bufs=6))

        idx_sb = const.tile([P, NBpad], mybir.dt.int32)
        dstoff_sb = const.tile([P, NBpad], f32)
        cval_sb = const.tile([P, NBpad], f32)
        att_sb = const.tile([P, NBpad], f32)
        iota_sb = const.tile([P, P], bf16)
        ones_sb = const.tile([P, 1], bf16)
        nc.sync.dma_start(idx_sb[:], idx_h[:])
        nc.sync.dma_start(dstoff_sb[:], dstoff_h[:])
        nc.sync.dma_start(cval_sb[:], cval_h[:])
        nc.sync.dma_start(iota_sb[:], iota_h[:])
        nc.sync.dma_start(ones_sb[:], ones_h[:])
        nc.scalar.activation(att_sb[:], cval_sb[:],
                             mybir.ActivationFunctionType.Sigmoid)

        chunk = 0
        for j in range(SLOTS):
            Cj = int(C[j])
            ps = pspool.tile([P, D + 1], f32, tag="ps")
            for t in range(Cj):
                gt = gpool.tile([P, D], bf16, tag="g")
                nc.gpsimd.indirect_dma_start(
                    out=gt[:],
                    out_offset=None,
                    in_=table[:],
                    in_offset=bass.IndirectOffsetOnAxis(
                        ap=idx_sb[:, chunk:chunk + 1], axis=0),
                )
                lh = lpool.tile([P, P], bf16, tag="lh")
                nc.vector.tensor_scalar(
                    lh[:], iota_sb[:],
                    dstoff_sb[:, chunk:chunk + 1],
                    att_sb[:, chunk:chunk + 1],
                    op0=mybir.AluOpType.is_equal,
                    op1=mybir.AluOpType.mult)
                nc.tensor.matmul(ps[:, 0:D], lh[:], gt[:],
                                 start=(t == 0), stop=False)
                nc.tensor.matmul(ps[:, D:D + 1], lh[:], ones_sb[:],
                                 start=False, stop=(t == Cj - 1))
                chunk += 1
            ot = opool.tile([P, D + 1], f32, tag="ot")
            nc.scalar.copy(ot[:], ps[:])
            nc.sync.dma_start(out_h[j], ot[:])
    nc.compile()
    return nc


def kernel(src_feat, dst_feat, att_w, att_b, edge_index, n_dst):
    from concourse.bass_utils import run_bass_kernel_spmd

    src_feat = np.asarray(src_feat, dtype=np.float32)
    dst_feat = np.asarray(dst_feat, dtype=np.float32)
    att_w = np.asarray(att_w, dtype=np.float32)
    att_b = np.asarray(att_b, dtype=np.float32)
    n_dst = int(n_dst)
    assert src_feat.shape == (N_NODES, D) and n_dst == N_NODES

    CHB = 8
    pre = _preprocess(src_feat, dst_feat, att_w, att_b, edge_index)
    in_maps, NBpad = _build_core_inputs(pre, src_feat, CHB)

    key = (tuple(pre["C"].tolist()), NBpad, CHB)
    if key not in _COMPILED:
        _COMPILED[key] = _build_kernel(pre["C"], NBpad, CHB)
    nc = _COMPILED[key]

    res = run_bass_kernel_spmd(nc, in_maps, core_ids=list(range(8)))
    outs = [res.results[c]["out"] for c in range(8)]  # [SLOTS,128,129] f32

    bin_of = pre["bin_of"]
    grp = pre["grp_of_bin"][bin_of]
    slot = pre["slot_of_bin"][bin_of]
    pos = pre["pos_of"]
    agg = np.empty((N_NODES, D), np.float32)
    cnt = np.empty(N_NODES, np.float32)
    for g in range(8):
        m = grp == g
        o = outs[g]
        agg[m] = o[slot[m], pos[m], :D]
        cnt[m] = o[slot[m], pos[m], D]
    cnt = np.maximum(cnt, np.float32(1e-8))
    return (agg / cnt[:, None]).astype(np.float32)
